# revision 92
# baseline (speedup 1.0000x reference)
"""Trainium2 Bass kernel for nn_Affinity (graph-matching affinity matrix).

Math per sample (validated against the reference):
  out[(a,c),(b,c')] = sum_{e2,e1} G2[a,e2] H2[b,e2] Me[e2,e1] G1[c,e1] H1[c,e1]
                      + diag(vec(Mp))

Key structural collapse (validated end-to-end in fp64 numpy):
  * The 1024x1024 output is a pure DOUBLE SCATTER of the 96x96 edge
    affinity matrix:  out[(a2[t],c1[e]), (b2[t],c'1[e])] = Me[t,e],
    plus diag(vec(Mp)).  (The reference's row-major flatten of Me pairs
    row-position t of the e1-enumeration with row-position t of the
    e2-enumeration; the placement below reproduces it exactly.)
  * Me = G1^T R + H1^T S with R|S = C1 G2 + C2 H2 | C2 G2 + C1 H2 and
    C_i = F1^T relu(l_i + l_i^T) F2 (32x32).  C1/C2 depend only on
    lambda/F, so all d=128 contractions run during the input-DMA window
    and concurrently with the rank chain; the adjacency-dependent path
    only does 32-contractions.

Device (1 sample per NeuronCore, fully static instruction stream):
  1. Packed input DMAs (lambda|A first, F|U second); select constants
     generated on device during the DMA window (DVE + Pool split).
  2. Row-major edge ranks via masked prefix-scan; one-hot rank
     expansion; G/H + per-edge endpoints via accumulating matmuls.
  3. C-chain on PE/Act in parallel with the rank chain.
  4. ONE output DMA: [Me | c | c' | a | b | MpT] (96 x 132 f16).  The
     host unshard only places device-computed values at device-computed
     indices (the Kronecker one-hot scatter) and casts.
"""

import numpy as np

import concourse.bacc as bacc
import concourse.bass as bass
import concourse.mybir as mybir
import concourse.tile as tile
from concourse.bass_utils import run_bass_kernel_spmd

F32 = mybir.dt.float32
F16 = mybir.dt.float16
I32 = mybir.dt.int32
ALU = mybir.AluOpType
AX = mybir.AxisListType
AF = mybir.ActivationFunctionType

B, N, D, E = 8, 32, 128, 96
NCORES = 8
OUTW = 132


def build_program(debug: bool = False):
    nc = bacc.Bacc("TRN2", target_bir_lowering=False, debug=debug,
                   num_devices=NCORES)
    # inp: lambda1 | lambda2 | A(16) | F1 | F2 | U1 | U2   (128, 400) f16
    inp = nc.dram_tensor("inp", [128, 400], F16, kind="ExternalInput")
    out_me = nc.dram_tensor("out_me", [96, OUTW], F16, kind="ExternalOutput")

    with tile.TileContext(nc) as tc:
        with tc.tile_pool(name="sb", bufs=1) as sb, \
             tc.tile_pool(name="ps", bufs=1, space="PSUM") as ps:
            in_sb = sb.tile([128, 400], F16, tag="in_sb")
            nc.sync.dma_start(out=in_sb[:, 0:272], in_=inp[:, 0:272])
            nc.sync.dma_start(out=in_sb[:, 272:400], in_=inp[:, 272:400])
            l1_16, l2_16 = in_sb[:, 0:128], in_sb[:, 128:256]
            a16 = in_sb[:, 256:272]             # A_src cols 0:8, A_tgt 8:16
            f1, f2 = in_sb[:, 272:304], in_sb[:, 304:336]
            u1, u2 = in_sb[:, 336:368], in_sb[:, 368:400]

            # dummy activation: absorbs the 1283ns act-table load up front
            # so the scheduler doesn't model the lamp relu as late
            scr = sb.tile([128, 2], F16, tag="scr")
            nc.scalar.memzero(scr[:])
            nc.scalar.activation(out=scr[:], in_=scr[:], func=AF.Relu)

            # ---- on-device constants (input-DMA window) ----
            it32 = sb.tile([128, 129], I32, tag="it32")
            nc.gpsimd.iota(it32[:], pattern=[[1, 129]], base=0,
                           channel_multiplier=0)
            pi32 = sb.tile([128, 1], I32, tag="pi32")
            nc.gpsimd.iota(pi32[:], pattern=[[1, 1]], base=0,
                           channel_multiplier=1)
            io16 = sb.tile([128, 129], F16, tag="io16")
            nc.vector.tensor_copy(out=io16[:], in_=it32[:])
            pf32 = sb.tile([128, 1], F32, tag="pf32")
            nc.vector.tensor_copy(out=pf32[:], in_=pi32[:])
            q32 = sb.tile([128, 1], I32, tag="q32")
            nc.vector.tensor_scalar(out=q32[:], in0=pi32[:], scalar1=2,
                                    scalar2=None,
                                    op0=ALU.logical_shift_right)
            qf32 = sb.tile([128, 1], F32, tag="qf32")
            nc.vector.tensor_copy(out=qf32[:], in_=q32[:])
            m32 = sb.tile([128, 1], I32, tag="m32")
            nc.vector.tensor_scalar(out=m32[:], in0=pi32[:], scalar1=3,
                                    scalar2=None, op0=ALU.bitwise_and)
            b832 = sb.tile([128, 1], I32, tag="b832")
            nc.vector.tensor_scalar(out=b832[:], in0=m32[:], scalar1=3,
                                    scalar2=None, op0=ALU.logical_shift_left)
            b8f = sb.tile([128, 1], F32, tag="b8f")
            nc.vector.tensor_copy(out=b8f[:], in_=b832[:])
            # hv16[:, 2k] = head value p//4; hv16[:, 2k+1] = tail 8(p%4)+k
            hv16 = sb.tile([128, 16], F16, tag="hv16")
            for k in range(8):
                nc.vector.tensor_copy(out=hv16[:, 2 * k:2 * k + 1],
                                      in_=qf32[:])
                nc.vector.tensor_scalar(out=hv16[:, 2 * k + 1:2 * k + 2],
                                        in0=b8f[:], scalar1=float(k),
                                        scalar2=None, op0=ALU.add)
            # hts: per-k stacked select [headsel | tailsel_k]  (128, 8*64)
            # head blocks in ONE strided DVE op; tail blocks on Pool
            hts = sb.tile([128, 512], F16, tag="hts")
            hts3 = hts[:].rearrange("p (k c) -> p k c", c=64)
            io_b = io16[:, 0:32].unsqueeze(1).broadcast_to([128, 8, 32])
            nc.vector.tensor_scalar(out=hts3[:, :, 0:32], in0=io_b,
                                    scalar1=qf32[:, 0:1], scalar2=None,
                                    op0=ALU.is_equal)
            # Pool-side bigger constants (after the iotas)
            id16 = sb.tile([128, 128], F16, tag="id16")
            nc.gpsimd.tensor_scalar(out=id16[:], in0=io16[:, 0:128],
                                    scalar1=pf32[:, 0:1], scalar2=None,
                                    op0=ALU.is_equal)
            su16 = sb.tile([128, 128], F16, tag="su16")
            nc.gpsimd.tensor_scalar(out=su16[:], in0=io16[:, 0:128],
                                    scalar1=pf32[:, 0:1], scalar2=None,
                                    op0=ALU.is_gt)
            for k in range(8):
                nc.gpsimd.tensor_scalar(out=hts[:, 64 * k + 32:64 * (k + 1)],
                                        in0=io16[:, 0:32],
                                        scalar1=b8f[:, 0:1],
                                        scalar2=float(k),
                                        op0=ALU.subtract, op1=ALU.is_equal)

            # ================= C-chain (PE + Act) =================
            lamp = ps.tile([128, 256], F32, tag="psL", bufs=1)
            for i, l_ in enumerate((l1_16, l2_16)):
                nc.tensor.matmul(out=lamp[:, 128 * i:128 * (i + 1)],
                                 lhsT=id16[:], rhs=l_,
                                 start=True, stop=False)
                nc.tensor.matmul(out=lamp[:, 128 * i:128 * (i + 1)],
                                 lhsT=l_, rhs=id16[:],
                                 start=False, stop=True)
            lp16 = sb.tile([128, 256], F16, tag="lp16")
            nc.scalar.activation(out=lp16[:], in_=lamp[:], func=AF.Relu)
            # B in both column orders: [B1|B2|B2|B1] so lhsT slices give
            # stacked [D1;D2] and [D2;D1] with no partition offsets
            bps = ps.tile([128, 128], F32, tag="psA", bufs=1)
            nc.tensor.matmul(out=bps[:, 0:32], lhsT=lp16[:, 0:128], rhs=f2,
                             start=True, stop=True)
            nc.tensor.matmul(out=bps[:, 32:64], lhsT=lp16[:, 128:256],
                             rhs=f2, start=True, stop=True)
            nc.tensor.matmul(out=bps[:, 64:96], lhsT=lp16[:, 128:256],
                             rhs=f2, start=True, stop=True)
            nc.tensor.matmul(out=bps[:, 96:128], lhsT=lp16[:, 0:128],
                             rhs=f2, start=True, stop=True)
            b16 = sb.tile([128, 128], F16, tag="b16")
            nc.scalar.copy(out=b16[:], in_=bps[:])
            # dstack = [[D1;D2] | [D2;D1]]  (64, 64)
            dps = ps.tile([64, 64], F32, tag="psL", bufs=1)
            nc.tensor.matmul(out=dps[:, 0:32], lhsT=b16[:, 0:64], rhs=f1,
                             start=True, stop=True)
            nc.tensor.matmul(out=dps[:, 32:64], lhsT=b16[:, 64:128], rhs=f1,
                             start=True, stop=True)
            d16 = sb.tile([64, 64], F16, tag="d16")
            nc.scalar.copy(out=d16[:], in_=dps[:])


            # ===== rank chain (f16 throughout; counts <= 96 are exact).
            # Scans run base-free the moment A lands (no PE round-trip in
            # front); row sums come from the scan tails; the partition
            # base is added afterwards, overlapping the pb matmul. =====
            s2 = sb.tile([128, 2], F16, tag="s2")
            r0 = sb.tile([128, 16], F16, tag="r0")
            r1h = sb.tile([128, 16], F32, tag="r1h")  # is_equal scalar: f32
            pb = ps.tile([128, 2], F32, tag="psI", bufs=1)
            with nc.allow_low_precision(reason="integer counts <= 96"):
                for g in (1, 0):
                    nc.vector.tensor_tensor_scan(
                        out=r0[:, 8 * g:8 * (g + 1)],
                        data0=a16[:, 8 * g:8 * (g + 1)],
                        data1=a16[:, 8 * g:8 * (g + 1)],
                        initial=0.0,
                        op0=ALU.add, op1=ALU.bypass)
                s2v = r0[:].rearrange("p (g k) -> p g k", k=8)[:, :, 7:8]
                nc.vector.tensor_copy(out=s2[:], in_=s2v.squeeze(2))
                nc.tensor.matmul(out=pb[:], lhsT=su16[:], rhs=s2[:],
                                 start=True, stop=True)
                for g in (1, 0):
                    nc.vector.tensor_scalar(out=r0[:, 8 * g:8 * (g + 1)],
                                            in0=r0[:, 8 * g:8 * (g + 1)],
                                            scalar1=pb[:, g:g + 1],
                                            scalar2=None, op0=ALU.add)
                    nc.vector.tensor_tensor(out=r1h[:, 8 * g:8 * (g + 1)],
                                            in0=r0[:, 8 * g:8 * (g + 1)],
                                            in1=a16[:, 8 * g:8 * (g + 1)],
                                            op=ALU.mult)

            # ---- graph2: one-hots + stacked [G2;H2] + endpoint matmuls ----
            caps = ps.tile([96, 36], F32, tag="psE", bufs=1)
            oh2t = sb.tile([128, 768], F16, tag="oh2")
            gh2ps = ps.tile([64, 96], F32, tag="psG2", bufs=1)
            for k in range(8):
                eng2 = nc.vector if k < 7 else nc.gpsimd
                eng2.tensor_scalar(
                    out=oh2t[:, 96 * k:96 * (k + 1)],
                    in0=io16[:, 1:97],
                    scalar1=r1h[:, 8 + k:9 + k], scalar2=None,
                    op0=ALU.is_equal)
            for k in range(8):
                nc.tensor.matmul(out=gh2ps[:], lhsT=hts[:, 64 * k:64 * (k + 1)],
                                 rhs=oh2t[:, 96 * k:96 * (k + 1)],
                                 start=(k == 0), stop=(k == 7))
            for k in range(8):
                nc.tensor.matmul(out=caps[:, 2:4],
                                 lhsT=oh2t[:, 96 * k:96 * (k + 1)],
                                 rhs=hv16[:, 2 * k:2 * (k + 1)],
                                 start=(k == 0), stop=(k == 7))
            gh2t = sb.tile([64, 96], F16, tag="gh2sb")
            nc.vector.tensor_copy(out=gh2t[:], in_=gh2ps[:])

            # ---- graph1: one-hots (k5-7 on Pool) + stacked [G1;H1] ----
            oh1t = sb.tile([128, 768], F16, tag="oh1")
            gh1ps = ps.tile([64, 96], F32, tag="psG1", bufs=1)
            for k in range(8):
                eng = nc.vector if k < 3 else nc.gpsimd
                eng.tensor_scalar(
                    out=oh1t[:, 96 * k:96 * (k + 1)],
                    in0=io16[:, 1:97],
                    scalar1=r1h[:, k:k + 1], scalar2=None,
                    op0=ALU.is_equal)
            for k in range(8):
                nc.tensor.matmul(out=gh1ps[:],
                                 lhsT=hts[:, 64 * k:64 * (k + 1)],
                                 rhs=oh1t[:, 96 * k:96 * (k + 1)],
                                 start=(k == 0), stop=(k == 7))
            for k in range(8):
                nc.tensor.matmul(out=caps[:, 0:2],
                                 lhsT=oh1t[:, 96 * k:96 * (k + 1)],
                                 rhs=hv16[:, 2 * k:2 * (k + 1)],
                                 start=(k == 0), stop=(k == 7))
            nc.tensor.matmul(out=caps[0:32, 4:36], lhsT=u2, rhs=u1,
                             start=True, stop=True)
            gh1t = sb.tile([64, 96], F16, tag="gh1sb")
            nc.vector.tensor_copy(out=gh1t[:], in_=gh1ps[:])

            # ---- [R;S] = d16^T @ [G2;H2]: ONE matmul, no offsets ----
            rsps = ps.tile([64, 96], F32, tag="psA", bufs=1)
            nc.tensor.matmul(out=rsps[:], lhsT=d16[:], rhs=gh2t[:],
                             start=True, stop=True)
            rs16 = sb.tile([64, 96], F16, tag="rs16")
            nc.scalar.copy(out=rs16[:], in_=rsps[:])

            # ---- Me = [G1;H1]^T [R;S]  (96, 96): ONE matmul ----
            meps = ps.tile([96, 96], F32, tag="psL", bufs=1)
            nc.tensor.matmul(out=meps[:], lhsT=gh1t[:], rhs=rs16[:],
                             start=True, stop=True)

            # ---- pack [Me | c | c' | a | b | MpT] and single DMA ----
            outsb = sb.tile([96, OUTW], F16, tag="outsb")
            nc.vector.tensor_copy(out=outsb[:, 96:OUTW], in_=caps[:])
            nc.vector.tensor_copy(out=outsb[:, 0:96], in_=meps[:])
            nc.sync.dma_start(out=out_me[:, :], in_=outsb[:, :])
    nc.compile()
    return nc


def make_in_maps(inputs: dict) -> list:
    inputs = {k: np.asarray(v, dtype=np.float32) for k, v in inputs.items()}
    in_maps = []
    for b in range(B):
        pk = np.zeros((128, 400), np.float16)
        pk[:, 0:128] = inputs["lambda1"]
        pk[:, 128:256] = inputs["lambda2"]
        pk[:, 256:264] = inputs["A_src"][b].reshape(128, 8)
        pk[:, 264:272] = inputs["A_tgt"][b].reshape(128, 8)
        pk[:, 272:304] = inputs["F_src"][b]
        pk[:, 304:336] = inputs["F_tgt"][b]
        pk[:, 336:368] = inputs["U_src"][b]
        pk[:, 368:400] = inputs["U_tgt"][b]
        in_maps.append({"inp": np.ascontiguousarray(pk)})
    return in_maps


_NC_CACHE = {}


def _assemble(res: dict) -> np.ndarray:
    """Place device-computed Me values at device-computed indices.

    out[(a2[t], c1[e]), (b2[t], c'1[e])] = Me[t, e]; out[i,i] += vec(Mp)[i].
    Pure placement + fp16->fp32 cast; no arithmetic on input data.
    """
    o = res["out_me"].astype(np.float32)
    me = o[:, 0:96]
    c = np.rint(o[:, 96]).astype(np.int64)
    cp = np.rint(o[:, 97]).astype(np.int64)
    a = np.rint(o[:, 98]).astype(np.int64)
    bb = np.rint(o[:, 99]).astype(np.int64)
    mpt = o[0:32, 100:132]                               # MpT[c, a]
    outm = np.zeros((1024, 1024), np.float32)
    o4 = outm.reshape(32, 32, 32, 32)
    o4[a[:, None], c[None, :], bb[:, None], cp[None, :]] = me
    outm[np.arange(1024), np.arange(1024)] += mpt.T.ravel()
    return outm


def kernel(trace: bool = False, **inputs) -> np.ndarray:
    if "nc" not in _NC_CACHE:
        _NC_CACHE["nc"] = build_program()
    nc = _NC_CACHE["nc"]
    in_maps = make_in_maps(inputs)
    res = run_bass_kernel_spmd(nc, in_maps, core_ids=list(range(NCORES)),
                               trace=trace)
    _NC_CACHE["last_results"] = res
    outs = [_assemble(res.results[b]) for b in range(B)]
    return np.stack(outs).astype(np.float32)
